# revision 11
# baseline (speedup 1.0000x reference)
"""Trainium2 Bass kernel for single-head dot-product self-attention.

  reference:  Q = x@Wq, K = x@Wk, V = x@Wv          (per batch element)
              out = softmax(Q K^T / sqrt(512)) @ V

Sharding: data-parallel over batch B=8 -> one batch element per NeuronCore.
All matmuls run in float32r (TF32-class rounding, IEEE fp32 container,
~2.4e-4 rel err per rounding, ~4x faster than fp32 on the PE).

Layout strategy per core (transposed-scores):
  - x [2048,512] is DMA'd naturally then PE-transposed once into
    xT [d, s] tiles (d on partitions): every projection contracts over d.
  - QT/KT are produced transposed ([u, s]); scores are computed
    TRANSPOSED: S^T[k, q] = KT_slice.T @ QT, k on PSUM partitions.
  - exp(S^T) tiles feed the PV matmul directly as the moving operand
    (stationary = natural-layout V slices), so no attention-matrix
    transposes are ever needed:  out^T[u, q] = sum_k V[k,u] expS^T[k,q].
  - softmax denominator: DVE accumulates expS^T tiles over k (free-axis
    q), one ones-vector matmul reduces the 128 partitions, and a 2KB
    DRAM round-trip redistributes den[1, 512] to per-partition [128, 4]
    for the final per-q scaling.
  - out^T is PE-transposed back to [q, u] (4x 128x128 per q-chunk into
    one PSUM bank), scaled by 1/den (per-partition scalar) and DMA'd out.

All tiles are 512 columns wide so cross-phase dependencies stay fine-
grained and the QKV and attention phases overlap on the scheduler.
"""

import sys

sys.path.insert(0, "/opt/trn_rl_repo")

import numpy as np

import concourse.bass as bass  # noqa: F401
import concourse.mybir as mybir
import concourse.tile as tile
from concourse import bacc
from concourse.bass_utils import run_bass_kernel_spmd
from concourse.masks import make_identity

f32 = mybir.dt.float32
f32r = mybir.dt.float32r

B, S, D, U = 8, 2048, 512, 512
P = 128                 # partitions
SC = S // P             # 16 s-chunks (also k-chunks)
DC = D // P             # 4 d-chunks
UC = U // P             # 4 u-chunks
NT = S // 512           # 4 512-wide tiles along s/k/q
SCALE = 1.0 / float(np.sqrt(U))
EXP = mybir.ActivationFunctionType.Exp


class _nullctx:
    def __enter__(self):
        return None

    def __exit__(self, *a):
        return False


def build_nc(repeat: int = 1, hw_all: int = 1):
    """repeat: python-unrolled whole-kernel reps (timing).
    hw_all: hardware-loop count around the whole kernel body (timing)."""
    nc = bacc.Bacc("TRN2", target_bir_lowering=False, debug=False)
    x_d = nc.dram_tensor("x", [S, D], f32, kind="ExternalInput")
    w_d = {
        "q": nc.dram_tensor("Wq", [D, U], f32, kind="ExternalInput"),
        "k": nc.dram_tensor("Wk", [D, U], f32, kind="ExternalInput"),
        "v": nc.dram_tensor("Wv", [D, U], f32, kind="ExternalInput"),
    }
    out_d = nc.dram_tensor("out", [S, U], f32, kind="ExternalOutput")
    scratch_d = nc.dram_tensor("den_scratch", [NT, 512], f32)

    with tile.TileContext(nc) as tc:
        with (
            tc.tile_pool(name="persist", bufs=1) as persist,
            tc.tile_pool(name="wstage", bufs=2) as wstage,
            tc.tile_pool(name="wr", bufs=1) as wr_pool,
            tc.tile_pool(name="xstage", bufs=4) as xstage,
            tc.tile_pool(name="xT", bufs=1) as xT_pool,
            tc.tile_pool(name="expp", bufs=1) as exp_pool,
            tc.tile_pool(name="attn_sb", bufs=2) as attn_pool,
            tc.tile_pool(name="outsb", bufs=2) as out_pool,
            tc.tile_pool(name="ps_mm", bufs=2, space="PSUM") as ps_mm,
            tc.tile_pool(name="ps_o", bufs=1, space="PSUM") as ps_o_pool,
            tc.tile_pool(name="ps_t", bufs=2, space="PSUM") as ps_t_pool,
        ):
            ident = persist.tile([P, P], f32, tag="ident")
            make_identity(nc, ident[:])
            ones_f = persist.tile([P, 1], f32, tag="ones_f")
            nc.gpsimd.memset(ones_f[:], 1.0)
            ones = persist.tile([P, 1], f32r, tag="ones")
            nc.vector.tensor_copy(ones[:], ones_f[:])

            # per-512-column tiles: [u][st] for QT/KT, [d][sg] for xT
            QT = [[persist.tile([P, 512], f32r, tag=f"QT{u}_{st}",
                                name=f"QT{u}_{st}") for st in range(NT)]
                  for u in range(UC)]
            KT = [[persist.tile([P, 512], f32r, tag=f"KT{u}_{st}",
                                name=f"KT{u}_{st}") for st in range(NT)]
                  for u in range(UC)]
            V = [persist.tile([P, U], f32r, tag=f"V{s}", name=f"V{s}")
                 for s in range(SC)]

            for _rep in range(repeat):
                with (tc.For_i(0, hw_all, 1) if hw_all > 1
                      else _nullctx()):
                    # ---------- phase 1+2: load, transpose x, project QKV
                    xs_all = []
                    for s in range(4):
                        t = xstage.tile([P, D], f32, tag="xs", name="xs")
                        nc.sync.dma_start(t[:], x_d[s * P:(s + 1) * P, :])
                        xs_all.append(t)
                    wr = {}
                    for wname, wd in w_d.items():
                        for d in range(DC):
                            stg = wstage.tile([P, U], f32, tag="wstg",
                                              name="wstg")
                            nc.sync.dma_start(stg[:], wd[d * P:(d + 1) * P, :])
                            wrt = wr_pool.tile([P, U], f32r,
                                               tag=f"w{wname}{d}",
                                               name=f"w{wname}{d}")
                            nc.scalar.copy(wrt[:], stg[:])
                            wr[wname, d] = wrt

                    for sg in range(NT):
                        xTg = [xT_pool.tile([P, 512], f32r, tag=f"xT{d}",
                                            name=f"xT{d}", bufs=2)
                               for d in range(DC)]
                        if sg == 0:
                            xs = xs_all
                        else:
                            xs = []
                            for j in range(4):
                                s = sg * 4 + j
                                t = xstage.tile([P, D], f32, tag="xs",
                                                name="xs")
                                nc.sync.dma_start(t[:],
                                                  x_d[s * P:(s + 1) * P, :])
                                xs.append(t)
                        for d in range(DC):
                            pst = ps_t_pool.tile([P, 512], f32, tag="t",
                                                 name="pst")
                            for j in range(4):
                                nc.tensor.transpose(
                                    pst[:, j * P:(j + 1) * P],
                                    xs[j][:, d * P:(d + 1) * P], ident[:])
                            nc.vector.tensor_copy(xTg[d][:], pst[:])

                        # projections for this 512-wide s window:
                        # QT/KT tiles [u][sg] and V s-chunks in the window
                        for wname, dstT in (("q", QT), ("k", KT)):
                            for u in range(UC):
                                ps = ps_mm.tile([P, 512], f32, tag="mm",
                                                name="ps")
                                for d in range(DC):
                                    nc.tensor.matmul(
                                        ps[:],
                                        wr[wname, d][:, u * P:(u + 1) * P],
                                        xTg[d][:],
                                        start=(d == 0), stop=(d == DC - 1))
                                nc.scalar.copy(dstT[u][sg][:], ps[:])
                        for j in range(4):
                            s = sg * 4 + j
                            ps = ps_mm.tile([P, 512], f32, tag="mm",
                                            name="ps")
                            for d in range(DC):
                                nc.tensor.matmul(
                                    ps[:], xTg[d][:, j * P:(j + 1) * P],
                                    wr["v", d][:],
                                    start=(d == 0), stop=(d == DC - 1))
                            nc.scalar.copy(V[s][:], ps[:])

                    # ---------- phase 3: attention per 512-wide q tile
                    for qt in range(NT):
                        den_acc = [attn_pool.tile([P, 512], f32,
                                                   tag=f"den_acc{h}",
                                                   name=f"den_acc{h}",
                                                   bufs=1)
                                   for h in range(2)]
                        den_accr0 = attn_pool.tile([P, 512], f32r,
                                                   tag="den_accr0",
                                                   name="den_accr0", bufs=1)
                        ps_den = ps_t_pool.tile([1, 512], f32, tag="t",
                                                name="ps_den")
                        ps_o = [ps_o_pool.tile([P, 512], f32, tag=f"o{c}",
                                               name=f"o{c}")
                                for c in range(4)]
                        for k in range(SC):
                            ps = ps_mm.tile([P, 512], f32, tag="mm",
                                            name="ps")
                            for u in range(UC):
                                nc.tensor.matmul(
                                    ps[:], KT[u][k // 4][:, (k % 4) * P:
                                                         (k % 4 + 1) * P],
                                    QT[u][qt][:],
                                    start=(u == 0), stop=(u == UC - 1))
                            e = exp_pool.tile([P, 512], f32r, tag="e",
                                              name="e", bufs=6)
                            nc.scalar.activation(e[:], ps[:], EXP,
                                                 scale=SCALE)
                            for c in range(4):
                                nc.tensor.matmul(
                                    ps_o[c][:],
                                    e[:, c * P:(c + 1) * P],
                                    V[k][:],
                                    start=(k == 0), stop=(k == SC - 1))
                            if k in (0, 8):
                                nc.vector.tensor_copy(den_acc[k // 8][:],
                                                      e[:].bitcast(f32))
                            else:
                                nc.vector.tensor_add(den_acc[k // 8][:],
                                                     den_acc[k // 8][:],
                                                     e[:].bitcast(f32))
                            if k == 7:
                                nc.vector.tensor_copy(den_accr0[:],
                                                      den_acc[0][:])
                                nc.tensor.matmul(ps_den[:], ones[:],
                                                 den_accr0[:],
                                                 start=True, stop=False)

                        # second-half partition reduction via ones-matmul
                        den_accr = attn_pool.tile([P, 512], f32r,
                                                  tag="den_accr",
                                                  name="den_accr", bufs=1)
                        nc.vector.tensor_copy(den_accr[:], den_acc[1][:])
                        nc.tensor.matmul(ps_den[:], ones[:], den_accr[:],
                                         start=False, stop=True)
                        den_sb = attn_pool.tile([1, 512], f32, tag="den_sb",
                                                name="den_sb")
                        nc.vector.tensor_copy(den_sb[:], ps_den[:])
                        # [1,512] -> [128,4] via 2KB DRAM round-trip
                        nc.sync.dma_start(scratch_d[qt, :], den_sb[:1, :])
                        denT = attn_pool.tile([P, NT], f32, tag="denT",
                                              name="denT")
                        nc.sync.dma_start(
                            denT[:],
                            scratch_d[qt, :].rearrange("(j p) -> p j", p=P))
                        recipT = attn_pool.tile([P, NT], f32,
                                                tag="recipT", name="recipT")
                        nc.vector.reciprocal(recipT[:], denT[:])

                        # scale rows by 1/den and store
                        for c in range(4):
                            outt = out_pool.tile([P, U], f32, tag="out",
                                                 name="outt")
                            nc.vector.tensor_scalar_mul(outt[:], ps_o[c][:],
                                                        recipT[:, c:c + 1])
                            q0 = qt * 512 + c * P
                            nc.sync.dma_start(out_d[q0:q0 + P, :], outt[:])

    nc.finalize()
    return nc


def kernel(x: np.ndarray, Wq: np.ndarray, Wk: np.ndarray,
           Wv: np.ndarray) -> np.ndarray:
    x = np.ascontiguousarray(x, dtype=np.float32)
    Wq = np.ascontiguousarray(Wq, dtype=np.float32)
    Wk = np.ascontiguousarray(Wk, dtype=np.float32)
    Wv = np.ascontiguousarray(Wv, dtype=np.float32)
    assert x.shape == (B, S, D)

    nc = build_nc()
    in_maps = [{"x": x[b], "Wq": Wq, "Wk": Wk, "Wv": Wv} for b in range(B)]
    res = run_bass_kernel_spmd(nc, in_maps, list(range(B)))
    return np.stack([res.results[b]["out"] for b in range(B)], axis=0)


if __name__ == "__main__":
    rng = np.random.default_rng(0)
    x = rng.standard_normal((B, S, D), dtype=np.float32)
    sc = 1.0 / np.sqrt(D)
    Wq = rng.standard_normal((D, U), dtype=np.float32) * sc
    Wk = rng.standard_normal((D, U), dtype=np.float32) * sc
    Wv = rng.standard_normal((D, U), dtype=np.float32) * sc
    out = kernel(x=x, Wq=Wq, Wk=Wk, Wv=Wv)
    print("out", out.shape, out.dtype)


# revision 12
# speedup vs baseline: 1.0731x; 1.0731x over previous
"""Trainium2 Bass kernel for single-head dot-product self-attention.

  reference:  Q = x@Wq, K = x@Wk, V = x@Wv          (per batch element)
              out = softmax(Q K^T / sqrt(512)) @ V

Sharding: data-parallel over batch B=8 -> one batch element per NeuronCore.
All matmuls run in float32r (TF32-class rounding, IEEE fp32 container,
~2.4e-4 rel err per rounding, ~4x faster than fp32 on the PE).

Layout strategy per core (transposed-scores):
  - x [2048,512] is DMA'd naturally then PE-transposed once into
    xT [d, s] tiles (d on partitions): every projection contracts over d.
  - QT/KT are produced transposed ([u, s]); scores are computed
    TRANSPOSED: S^T[k, q] = KT_slice.T @ QT, k on PSUM partitions.
  - exp(S^T) tiles feed the PV matmul directly as the moving operand
    (stationary = natural-layout V slices), so no attention-matrix
    transposes are ever needed:  out^T[u, q] = sum_k V[k,u] expS^T[k,q].
  - softmax denominator: DVE accumulates expS^T tiles over k (free-axis
    q), one ones-vector matmul reduces the 128 partitions, and a 2KB
    DRAM round-trip redistributes den[1, 512] to per-partition [128, 4]
    for the final per-q scaling.
  - out^T is PE-transposed back to [q, u] (4x 128x128 per q-chunk into
    one PSUM bank), scaled by 1/den (per-partition scalar) and DMA'd out.

All tiles are 512 columns wide so cross-phase dependencies stay fine-
grained and the QKV and attention phases overlap on the scheduler.
"""

import sys

sys.path.insert(0, "/opt/trn_rl_repo")

import numpy as np

import concourse.bass as bass  # noqa: F401
import concourse.mybir as mybir
import concourse.tile as tile
from concourse import bacc
from concourse.bass_utils import run_bass_kernel_spmd
from concourse.masks import make_identity

f32 = mybir.dt.float32
f32r = mybir.dt.float32r

B, S, D, U = 8, 2048, 512, 512
P = 128                 # partitions
SC = S // P             # 16 s-chunks (also k-chunks)
DC = D // P             # 4 d-chunks
UC = U // P             # 4 u-chunks
NT = S // 512           # 4 512-wide tiles along s/k/q
SCALE = 1.0 / float(np.sqrt(U))
EXP = mybir.ActivationFunctionType.Exp


class _nullctx:
    def __enter__(self):
        return None

    def __exit__(self, *a):
        return False


def build_nc(repeat: int = 1, hw_all: int = 1):
    """repeat: python-unrolled whole-kernel reps (timing).
    hw_all: hardware-loop count around the whole kernel body (timing)."""
    nc = bacc.Bacc("TRN2", target_bir_lowering=False, debug=False)
    x_d = nc.dram_tensor("x", [S, D], f32, kind="ExternalInput")
    w_d = {
        "q": nc.dram_tensor("Wq", [D, U], f32, kind="ExternalInput"),
        "k": nc.dram_tensor("Wk", [D, U], f32, kind="ExternalInput"),
        "v": nc.dram_tensor("Wv", [D, U], f32, kind="ExternalInput"),
    }
    out_d = nc.dram_tensor("out", [S, U], f32, kind="ExternalOutput")
    scratch_d = nc.dram_tensor("den_scratch", [NT, 512], f32)

    with tile.TileContext(nc) as tc:
        with (
            tc.tile_pool(name="persist", bufs=1) as persist,
            tc.tile_pool(name="wstage", bufs=2) as wstage,
            tc.tile_pool(name="wr", bufs=1) as wr_pool,
            tc.tile_pool(name="xstage", bufs=8) as xstage,
            tc.tile_pool(name="xT", bufs=1) as xT_pool,
            tc.tile_pool(name="expp", bufs=1) as exp_pool,
            tc.tile_pool(name="attn_sb", bufs=2) as attn_pool,
            tc.tile_pool(name="outsb", bufs=2) as out_pool,
            tc.tile_pool(name="ps_mm", bufs=2, space="PSUM") as ps_mm,
            tc.tile_pool(name="ps_o", bufs=1, space="PSUM") as ps_o_pool,
            tc.tile_pool(name="ps_t", bufs=2, space="PSUM") as ps_t_pool,
        ):
            ident = persist.tile([P, P], f32, tag="ident")
            make_identity(nc, ident[:])
            ones_f = persist.tile([P, 1], f32, tag="ones_f")
            nc.gpsimd.memset(ones_f[:], 1.0)
            ones = persist.tile([P, 1], f32r, tag="ones")
            nc.vector.tensor_copy(ones[:], ones_f[:])

            # per-512-column tiles: [u][st] for QT/KT, [d][sg] for xT
            QT = [[persist.tile([P, 512], f32r, tag=f"QT{u}_{st}",
                                name=f"QT{u}_{st}") for st in range(NT)]
                  for u in range(UC)]
            KT = [[persist.tile([P, 512], f32r, tag=f"KT{u}_{st}",
                                name=f"KT{u}_{st}") for st in range(NT)]
                  for u in range(UC)]
            V = [persist.tile([P, U], f32r, tag=f"V{s}", name=f"V{s}")
                 for s in range(SC)]

            for _rep in range(repeat):
                with (tc.For_i(0, hw_all, 1) if hw_all > 1
                      else _nullctx()):
                    # ---------- phase 1+2: load, transpose x, project QKV
                    xs_all = []
                    for s in range(4):
                        t = xstage.tile([P, D], f32, tag="xs", name="xs")
                        nc.sync.dma_start(t[:], x_d[s * P:(s + 1) * P, :])
                        xs_all.append(t)
                    wr = {}
                    for wname, wd in w_d.items():
                        for d in range(DC):
                            stg = wstage.tile([P, U], f32, tag="wstg",
                                              name="wstg")
                            nc.sync.dma_start(stg[:], wd[d * P:(d + 1) * P, :])
                            wrt = wr_pool.tile([P, U], f32r,
                                               tag=f"w{wname}{d}",
                                               name=f"w{wname}{d}")
                            nc.scalar.copy(wrt[:], stg[:])
                            wr[wname, d] = wrt

                    for sg in range(NT):
                        xTg = [xT_pool.tile([P, 512], f32r, tag=f"xT{d}",
                                            name=f"xT{d}", bufs=2)
                               for d in range(DC)]
                        if sg == 0:
                            xs = xs_all
                        else:
                            xs = []
                            for j in range(4):
                                s = sg * 4 + j
                                t = xstage.tile([P, D], f32, tag="xs",
                                                name="xs")
                                nc.sync.dma_start(t[:],
                                                  x_d[s * P:(s + 1) * P, :])
                                xs.append(t)
                        for d in range(DC):
                            pst = ps_t_pool.tile([P, 512], f32, tag="t",
                                                 name="pst")
                            for j in range(4):
                                nc.tensor.transpose(
                                    pst[:, j * P:(j + 1) * P],
                                    xs[j][:, d * P:(d + 1) * P], ident[:])
                            nc.vector.tensor_copy(xTg[d][:], pst[:])

                        # projections for this 512-wide s window:
                        # QT/KT tiles [u][sg] and V s-chunks in the window
                        for wname, dstT in (("q", QT), ("k", KT)):
                            for u in range(UC):
                                ps = ps_mm.tile([P, 512], f32, tag="mm",
                                                name="ps")
                                for d in range(DC):
                                    nc.tensor.matmul(
                                        ps[:],
                                        wr[wname, d][:, u * P:(u + 1) * P],
                                        xTg[d][:],
                                        start=(d == 0), stop=(d == DC - 1))
                                nc.scalar.copy(dstT[u][sg][:], ps[:])
                        for j in range(4):
                            s = sg * 4 + j
                            ps = ps_mm.tile([P, 512], f32, tag="mm",
                                            name="ps")
                            for d in range(DC):
                                nc.tensor.matmul(
                                    ps[:], xTg[d][:, j * P:(j + 1) * P],
                                    wr["v", d][:],
                                    start=(d == 0), stop=(d == DC - 1))
                            nc.scalar.copy(V[s][:], ps[:])

                    # ---------- phase 3: attention per 512-wide q tile
                    for qt in range(NT):
                        den_acc = [attn_pool.tile([P, 512], f32,
                                                   tag=f"den_acc{h}",
                                                   name=f"den_acc{h}",
                                                   bufs=1)
                                   for h in range(2)]
                        den_accr0 = attn_pool.tile([P, 512], f32r,
                                                   tag="den_accr0",
                                                   name="den_accr0", bufs=1)
                        ps_den = ps_t_pool.tile([1, 512], f32, tag="t",
                                                name="ps_den")
                        ps_o = [ps_o_pool.tile([P, 512], f32, tag=f"o{c}",
                                               name=f"o{c}")
                                for c in range(4)]
                        for k in range(SC):
                            ps = ps_mm.tile([P, 512], f32, tag="mm",
                                            name="ps")
                            for u in range(UC):
                                nc.tensor.matmul(
                                    ps[:], KT[u][k // 4][:, (k % 4) * P:
                                                         (k % 4 + 1) * P],
                                    QT[u][qt][:],
                                    start=(u == 0), stop=(u == UC - 1))
                            e = exp_pool.tile([P, 512], f32r, tag="e",
                                              name="e", bufs=6)
                            nc.scalar.activation(e[:], ps[:], EXP,
                                                 scale=SCALE)
                            for c in range(4):
                                nc.tensor.matmul(
                                    ps_o[c][:],
                                    e[:, c * P:(c + 1) * P],
                                    V[k][:],
                                    start=(k == 0), stop=(k == SC - 1))
                            if k in (0, 8):
                                nc.vector.tensor_copy(den_acc[k // 8][:],
                                                      e[:].bitcast(f32))
                            else:
                                nc.vector.tensor_add(den_acc[k // 8][:],
                                                     den_acc[k // 8][:],
                                                     e[:].bitcast(f32))
                            if k == 7:
                                nc.vector.tensor_copy(den_accr0[:],
                                                      den_acc[0][:])
                                nc.tensor.matmul(ps_den[:], ones[:],
                                                 den_accr0[:],
                                                 start=True, stop=False)

                        # second-half partition reduction via ones-matmul
                        den_accr = attn_pool.tile([P, 512], f32r,
                                                  tag="den_accr",
                                                  name="den_accr", bufs=1)
                        nc.vector.tensor_copy(den_accr[:], den_acc[1][:])
                        nc.tensor.matmul(ps_den[:], ones[:], den_accr[:],
                                         start=False, stop=True)
                        den_sb = attn_pool.tile([1, 512], f32, tag="den_sb",
                                                name="den_sb")
                        nc.vector.tensor_copy(den_sb[:], ps_den[:])
                        # [1,512] -> [128,4] via 2KB DRAM round-trip
                        nc.sync.dma_start(scratch_d[qt, :], den_sb[:1, :])
                        denT = attn_pool.tile([P, NT], f32, tag="denT",
                                              name="denT")
                        nc.sync.dma_start(
                            denT[:],
                            scratch_d[qt, :].rearrange("(j p) -> p j", p=P))
                        recipT = attn_pool.tile([P, NT], f32,
                                                tag="recipT", name="recipT")
                        nc.vector.reciprocal(recipT[:], denT[:])

                        # scale rows by 1/den and store
                        for c in range(4):
                            outt = out_pool.tile([P, U], f32, tag="out",
                                                 name="outt")
                            nc.vector.tensor_scalar_mul(outt[:], ps_o[c][:],
                                                        recipT[:, c:c + 1])
                            q0 = qt * 512 + c * P
                            nc.sync.dma_start(out_d[q0:q0 + P, :], outt[:])

    nc.finalize()
    return nc


def kernel(x: np.ndarray, Wq: np.ndarray, Wk: np.ndarray,
           Wv: np.ndarray) -> np.ndarray:
    x = np.ascontiguousarray(x, dtype=np.float32)
    Wq = np.ascontiguousarray(Wq, dtype=np.float32)
    Wk = np.ascontiguousarray(Wk, dtype=np.float32)
    Wv = np.ascontiguousarray(Wv, dtype=np.float32)
    assert x.shape == (B, S, D)

    nc = build_nc()
    in_maps = [{"x": x[b], "Wq": Wq, "Wk": Wk, "Wv": Wv} for b in range(B)]
    res = run_bass_kernel_spmd(nc, in_maps, list(range(B)))
    return np.stack([res.results[b]["out"] for b in range(B)], axis=0)


if __name__ == "__main__":
    rng = np.random.default_rng(0)
    x = rng.standard_normal((B, S, D), dtype=np.float32)
    sc = 1.0 / np.sqrt(D)
    Wq = rng.standard_normal((D, U), dtype=np.float32) * sc
    Wk = rng.standard_normal((D, U), dtype=np.float32) * sc
    Wv = rng.standard_normal((D, U), dtype=np.float32) * sc
    out = kernel(x=x, Wq=Wq, Wk=Wk, Wv=Wv)
    print("out", out.shape, out.dtype)


# revision 16
# speedup vs baseline: 1.0996x; 1.0247x over previous
"""Trainium2 Bass kernel for single-head dot-product self-attention.

  reference:  Q = x@Wq, K = x@Wk, V = x@Wv          (per batch element)
              out = softmax(Q K^T / sqrt(512)) @ V

Sharding: data-parallel over batch B=8 -> one batch element per NeuronCore.
All matmuls run in float32r (TF32-class rounding, IEEE fp32 container,
~2.4e-4 rel err per rounding, ~4x faster than fp32 on the PE).

Layout strategy per core (transposed-scores):
  - x [2048,512] is DMA'd naturally then PE-transposed once into
    xT [d, s] tiles (d on partitions): every projection contracts over d.
  - QT/KT are produced transposed ([u, s]); scores are computed
    TRANSPOSED: S^T[k, q] = KT_slice.T @ QT, k on PSUM partitions.
  - exp(S^T) tiles feed the PV matmul directly as the moving operand
    (stationary = natural-layout V slices), so no attention-matrix
    transposes are ever needed:  out^T[u, q] = sum_k V[k,u] expS^T[k,q].
  - softmax denominator: DVE accumulates expS^T tiles over k (free-axis
    q), one ones-vector matmul reduces the 128 partitions, and a 2KB
    DRAM round-trip redistributes den[1, 512] to per-partition [128, 4]
    for the final per-q scaling.
  - out^T is PE-transposed back to [q, u] (4x 128x128 per q-chunk into
    one PSUM bank), scaled by 1/den (per-partition scalar) and DMA'd out.

All tiles are 512 columns wide so cross-phase dependencies stay fine-
grained and the QKV and attention phases overlap on the scheduler.
"""

import sys

sys.path.insert(0, "/opt/trn_rl_repo")

import numpy as np

import concourse.bass as bass  # noqa: F401
import concourse.mybir as mybir
import concourse.tile as tile
from concourse import bacc
from concourse.bass_utils import run_bass_kernel_spmd
from concourse.masks import make_identity

f32 = mybir.dt.float32
f32r = mybir.dt.float32r

B, S, D, U = 8, 2048, 512, 512
P = 128                 # partitions
SC = S // P             # 16 s-chunks (also k-chunks)
DC = D // P             # 4 d-chunks
UC = U // P             # 4 u-chunks
NT = S // 512           # 4 512-wide tiles along s/k/q
SCALE = 1.0 / float(np.sqrt(U))
EXP = mybir.ActivationFunctionType.Exp


class _nullctx:
    def __enter__(self):
        return None

    def __exit__(self, *a):
        return False


def build_nc(repeat: int = 1, hw_all: int = 1):
    """repeat: python-unrolled whole-kernel reps (timing).
    hw_all: hardware-loop count around the whole kernel body (timing)."""
    nc = bacc.Bacc("TRN2", target_bir_lowering=False, debug=False)
    x_d = nc.dram_tensor("x", [S, D], f32, kind="ExternalInput")
    w_d = {
        "q": nc.dram_tensor("Wq", [D, U], f32, kind="ExternalInput"),
        "k": nc.dram_tensor("Wk", [D, U], f32, kind="ExternalInput"),
        "v": nc.dram_tensor("Wv", [D, U], f32, kind="ExternalInput"),
    }
    out_d = nc.dram_tensor("out", [S, U], f32, kind="ExternalOutput")

    with tile.TileContext(nc) as tc:
        with (
            tc.tile_pool(name="persist", bufs=1) as persist,
            tc.tile_pool(name="wstage", bufs=2) as wstage,
            tc.tile_pool(name="wr", bufs=1) as wr_pool,
            tc.tile_pool(name="xstage", bufs=8) as xstage,
            tc.tile_pool(name="xT", bufs=1) as xT_pool,
            tc.tile_pool(name="expp", bufs=1) as exp_pool,
            tc.tile_pool(name="attn_sb", bufs=2) as attn_pool,
            tc.tile_pool(name="outsb", bufs=4) as out_pool,
            tc.tile_pool(name="ps_mm", bufs=2, space="PSUM") as ps_mm,
            tc.tile_pool(name="ps_o", bufs=1, space="PSUM") as ps_o_pool,
            tc.tile_pool(name="ps_t", bufs=2, space="PSUM") as ps_t_pool,
        ):
            ident = persist.tile([P, P], f32, tag="ident")
            make_identity(nc, ident[:])
            ones_f = persist.tile([P, 1], f32, tag="ones_f")
            nc.gpsimd.memset(ones_f[:], 1.0)
            ones = persist.tile([P, 1], f32r, tag="ones")
            nc.vector.tensor_copy(ones[:], ones_f[:])

            # per-512-column tiles: [u][st] for QT/KT, [d][sg] for xT
            QT = [[persist.tile([P, 512], f32r, tag=f"QT{u}_{st}",
                                name=f"QT{u}_{st}") for st in range(NT)]
                  for u in range(UC)]
            KT = [[persist.tile([P, 512], f32r, tag=f"KT{u}_{st}",
                                name=f"KT{u}_{st}") for st in range(NT)]
                  for u in range(UC)]
            V = [persist.tile([P, U], f32r, tag=f"V{s}", name=f"V{s}")
                 for s in range(SC)]

            for _rep in range(repeat):
                with (tc.For_i(0, hw_all, 1) if hw_all > 1
                      else _nullctx()):
                    # ---------- phase 1+2: load, transpose x, project QKV
                    xs_all = []
                    for s in range(4):
                        t = xstage.tile([P, D], f32, tag="xs", name="xs")
                        nc.sync.dma_start(t[:], x_d[s * P:(s + 1) * P, :])
                        xs_all.append(t)
                    wr = {}
                    for wname, wd in w_d.items():
                        for d in range(DC):
                            stg = wstage.tile([P, U], f32, tag="wstg",
                                              name="wstg")
                            nc.sync.dma_start(stg[:], wd[d * P:(d + 1) * P, :])
                            wrt = wr_pool.tile([P, U], f32r,
                                               tag=f"w{wname}{d}",
                                               name=f"w{wname}{d}")
                            if wname == "q" or (wname == "v" and d % 2):
                                nc.scalar.copy(wrt[:], stg[:])
                            else:
                                nc.vector.tensor_copy(wrt[:], stg[:])
                            wr[wname, d] = wrt

                    for sg in range(NT):
                        xTg = [xT_pool.tile([P, 512], f32r, tag=f"xT{d}",
                                            name=f"xT{d}", bufs=2)
                               for d in range(DC)]
                        if sg == 0:
                            xs = xs_all
                        else:
                            xs = []
                            for j in range(4):
                                s = sg * 4 + j
                                t = xstage.tile([P, D], f32, tag="xs",
                                                name="xs")
                                nc.sync.dma_start(t[:],
                                                  x_d[s * P:(s + 1) * P, :])
                                xs.append(t)
                        for d in range(DC):
                            pst = ps_t_pool.tile([P, 512], f32, tag="t",
                                                 name="pst")
                            for j in range(4):
                                nc.tensor.transpose(
                                    pst[:, j * P:(j + 1) * P],
                                    xs[j][:, d * P:(d + 1) * P], ident[:])
                            nc.vector.tensor_copy(xTg[d][:], pst[:])

                        # projections for this 512-wide s window:
                        # QT/KT tiles [u][sg] and V s-chunks in the window
                        for wname, dstT in (("q", QT), ("k", KT)):
                            for u in range(UC):
                                ps = ps_mm.tile([P, 512], f32, tag="mm",
                                                name="ps")
                                for d in range(DC):
                                    nc.tensor.matmul(
                                        ps[:],
                                        wr[wname, d][:, u * P:(u + 1) * P],
                                        xTg[d][:],
                                        start=(d == 0), stop=(d == DC - 1))
                                nc.scalar.copy(dstT[u][sg][:], ps[:])
                        for j in range(4):
                            s = sg * 4 + j
                            ps = ps_mm.tile([P, 512], f32, tag="mm",
                                            name="ps")
                            for d in range(DC):
                                nc.tensor.matmul(
                                    ps[:], xTg[d][:, j * P:(j + 1) * P],
                                    wr["v", d][:],
                                    start=(d == 0), stop=(d == DC - 1))
                            nc.scalar.copy(V[s][:], ps[:])

                    # ---------- phase 3: attention per 512-wide q tile
                    for qt in range(NT):
                        den_acc = attn_pool.tile([P, 512], f32,
                                                  tag="den_acc",
                                                  name="den_acc", bufs=1)
                        ps_den = ps_t_pool.tile([P, 512], f32, tag="t",
                                                name="ps_den")
                        ps_o = [ps_o_pool.tile([P, 512], f32, tag=f"o{c}",
                                               name=f"o{c}")
                                for c in range(4)]
                        for k in range(SC):
                            ps = ps_mm.tile([P, 512], f32, tag="mm",
                                            name="ps")
                            for u in range(UC):
                                nc.tensor.matmul(
                                    ps[:], KT[u][k // 4][:, (k % 4) * P:
                                                         (k % 4 + 1) * P],
                                    QT[u][qt][:],
                                    start=(u == 0), stop=(u == UC - 1))
                            e = exp_pool.tile([P, 512], f32r, tag="e",
                                              name="e", bufs=6)
                            nc.scalar.activation(e[:], ps[:], EXP,
                                                 scale=SCALE)
                            for c in range(4):
                                nc.tensor.matmul(
                                    ps_o[c][:],
                                    e[:, c * P:(c + 1) * P],
                                    V[k][:],
                                    start=(k == 0), stop=(k == SC - 1))
                            if k == 0:
                                nc.vector.tensor_copy(den_acc[:],
                                                      e[:].bitcast(f32))
                            else:
                                nc.vector.tensor_add(den_acc[:], den_acc[:],
                                                     e[:].bitcast(f32))

                        # partition reduction via 4 fp32 matvecs; result
                        # lands directly in [q-partition, chunk] layout.
                        for c in range(4):
                            nc.tensor.matmul(
                                ps_den[:, c:c + 1],
                                den_acc[:, c * P:(c + 1) * P],
                                ones_f[:], start=True, stop=True)
                        recipT = attn_pool.tile([P, NT], f32,
                                                tag="recipT", name="recipT")
                        nc.vector.reciprocal(recipT[:], ps_den[:, 0:NT])

                        # scale rows by 1/den and store
                        for c in range(4):
                            outt = out_pool.tile([P, U], f32, tag="out",
                                                 name="outt")
                            nc.vector.tensor_scalar_mul(
                                outt[:], ps_o[c][:], recipT[:, c:c + 1])
                            q0 = qt * 512 + c * P
                            nc.sync.dma_start(out_d[q0:q0 + P, :], outt[:])

    nc.finalize()
    return nc


def kernel(x: np.ndarray, Wq: np.ndarray, Wk: np.ndarray,
           Wv: np.ndarray) -> np.ndarray:
    x = np.ascontiguousarray(x, dtype=np.float32)
    Wq = np.ascontiguousarray(Wq, dtype=np.float32)
    Wk = np.ascontiguousarray(Wk, dtype=np.float32)
    Wv = np.ascontiguousarray(Wv, dtype=np.float32)
    assert x.shape == (B, S, D)

    nc = build_nc()
    in_maps = [{"x": x[b], "Wq": Wq, "Wk": Wk, "Wv": Wv} for b in range(B)]
    res = run_bass_kernel_spmd(nc, in_maps, list(range(B)))
    return np.stack([res.results[b]["out"] for b in range(B)], axis=0)


if __name__ == "__main__":
    rng = np.random.default_rng(0)
    x = rng.standard_normal((B, S, D), dtype=np.float32)
    sc = 1.0 / np.sqrt(D)
    Wq = rng.standard_normal((D, U), dtype=np.float32) * sc
    Wk = rng.standard_normal((D, U), dtype=np.float32) * sc
    Wv = rng.standard_normal((D, U), dtype=np.float32) * sc
    out = kernel(x=x, Wq=Wq, Wk=Wk, Wv=Wv)
    print("out", out.shape, out.dtype)
